# revision 24
# baseline (speedup 1.0000x reference)
"""Multi-head causal attention (B=2, T=2048, D=1024, H=16) on 8 Trainium2
NeuronCores.

Sharding: batch x head-group data/tensor parallel. Core c handles batch
c//4 and heads (c%4)*4 .. +4: W_qkv is split column-wise per head group,
W_o row-wise; each core computes attention for its local heads and a
partial output projection. The host sums the 4 partials per batch
(row-parallel W_o reduction) and stacks the two batches.

Per-core device kernel (fp16 data path, fp32 PSUM accumulate):
  Software-pipelined over q-chunks j=0..3; for each j:
    - projection slice: qkT[:, j*512:+512] = Wqk.T @ xT (per-head Q tiles
      with zeroed partition rows 64-127 and K tiles with finite partner
      rows, so the QK matmul runs with a full K=128 contraction);
    - V k-tiles 4j..4j+3 in natural layout with a per-head ones column
      (the ones column makes the AV matmul also emit the softmax
      denominator row);
    - attention for all 4 heads at chunk j: S.T = KT.T @ QT (PE) ->
      exp(s/8) (ACT, PSUM->SBUF fp16) -> causal-mask multiply on
      diagonal-crossing tiles (DVE) -> AV accumulate [65,512] (PE).
      Normalization is deferred one head pair: denominator rows are
      copied to partitions 0/32 of a [33,512] tile, 1/d computed on ACT
      as exp(-ln d) (same table set as the softmax exps), broadcast via a
      K=1 outer-product matmul (PE), multiplied into attnT (fp32r).
  The W_o projection (fp32r) for chunk j is interleaved into section j+1,
  streaming partial_out rows as fp16.

Softmax skips the max-subtraction: scores are ~N(0,1) after the 1/8 scale,
so exp never overflows fp32 and matches jax.nn.softmax to ~1e-6.
"""
import sys

for _p in ("/opt/trn_rl_repo", "/root/.axon_site/_ro/trn_rl_repo"):
    if _p not in sys.path:
        sys.path.insert(0, _p)

import numpy as np
import concourse.bass as bass
import concourse.mybir as mybir
import concourse.tile as tile
from concourse.vector_clock import ScopedClock
from concourse.bass_utils import run_bass_kernel_spmd

F32 = mybir.dt.float32
F32R = mybir.dt.float32r
F16 = mybir.dt.float16
AF = mybir.ActivationFunctionType

B, T, D = 2, 2048, 1024
N_CORES = 8
HPC = 4            # heads per core
HL = HPC * 64      # 256 local head dims
NKT = T // 128     # 16 k-tiles per head
NQC = T // 512     # 4 q-chunks


class FixedTileContext(tile.TileContext):
    """Works around this walrus build's 1-sync-wait-per-instruction limit.

    1. `_add_instruction`: peel extra waits off any instruction onto
       standalone single-wait nops emitted just before it on the same
       engine (the sequencer executes them in order).
    2. `_drain_and_barrier`: replace the tail drain (which carries one wait
       per outstanding proc) with chained single-wait sync-engine nops
       followed by a wait-free drain.
    """

    def _add_instruction(self, inst):
        si = inst.sync_info
        if si is not None:
            waits = list(si.on_wait)
            if len(waits) > 1:
                eng = getattr(inst, "engine", None)
                eng_obj = self.nc.engines.get(eng) if eng is not None else None
                if eng_obj is not None:
                    for w in waits[:-1]:
                        nop = eng_obj.nop()
                        nop.ins.sync_info = mybir.SyncInfo(on_wait=[w], on_update=[])
                    inst.sync_info = mybir.SyncInfo(
                        on_wait=[waits[-1]], on_update=list(si.on_update)
                    )
        super()._add_instruction(inst)

    def _drain_and_barrier(self, tick_clock, wait_clock):
        vec = tick_clock.global_clock
        for proc in range(len(vec)):
            t = vec[proc]
            if t <= 0:
                continue
            partial = ScopedClock()
            partial.require_at_least(None, proc, t)
            w = self.nc.sync.nop()
            wait_clock.add_sem_waits(w.ins, partial)
        self.nc.sync.drain()
        self.nc.all_engine_barrier()
        assert self.sems is not None
        popped = self.nc._tile_sem_poison_stack.pop()
        assert popped is self._sem_poison
        self.nc.clear_and_free_semaphores(list(self.sems.allocated().values()))
        self.nc.all_engine_barrier()


def build_nc():
    nc = bass.Bass()
    cx = nc.declare_dram_parameter("cx", [D, 2816], F16, isOutput=False)
    wo = nc.declare_dram_parameter("wo", [HL, D], F32R, isOutput=False)
    consts = nc.declare_dram_parameter("consts", [128, 128], F16, isOutput=False)
    out = nc.declare_dram_parameter("out", [T, D], F16, isOutput=True)

    with FixedTileContext(nc) as tc:
        with tc.tile_pool(name="persist", bufs=1) as pp, \
             tc.tile_pool(name="work", bufs=8) as wp, \
             tc.tile_pool(name="nwork", bufs=4) as nwp, \
             tc.tile_pool(name="psum", bufs=2, space="PSUM") as psp:
            consts_t = pp.tile([128, 128], F16, tag="consts")
            nc.sync.dma_start(consts_t[:], consts[:])
            ones_t = pp.tile([128, 64], F16, tag="ones")
            nc.gpsimd.memset(ones_t[:], 1.0)
            wo_t = []
            for c in range(2):
                w = pp.tile([128, D], F32R, tag=f"wo{c}", name=f"wo{c}")
                nc.sync.dma_start(w[:], wo[c * 128:(c + 1) * 128, :])
                wo_t.append(w)

            # comb layout [Wqk 512 | Wv 256 | xT 2048]; DMAs split so the
            # weights and the first token chunk land before later chunks
            comb = []
            bounds = [0, 768, 1792, 2816]
            for k in range(8):
                ct = pp.tile([128, 2816], F16, tag=f"comb{k}", name=f"comb{k}")
                for b0, b1 in zip(bounds, bounds[1:]):
                    nc.sync.dma_start(ct[:, b0:b1], cx[k * 128:(k + 1) * 128, b0:b1])
                comb.append(ct)

            # per-head Q and K tiles [128, T]; Q rows 64-127 zeroed
            q_t, k_t = [], []
            for h in range(HPC):
                qt = pp.tile([128, T], F16, tag=f"q{h}", name=f"q{h}")
                nc.gpsimd.memset(qt[64:128, :], 0.0)
                q_t.append(qt)
                kt = pp.tile([128, T], F16, tag=f"k{h}", name=f"k{h}")
                k_t.append(kt)
            vp_t = [pp.tile([128, HPC * 65], F16, tag=f"v{i}", name=f"v{i}")
                    for i in range(NKT)]
            at_t = [pp.tile([128, T], F32R, tag=f"at{c}", name=f"at{c}")
                    for c in range(2)]

            def proj_group(j, m):
                # qkT[:, j-chunk]: m=0,1 -> Q heads (2m, 2m+1); m=2,3 -> K
                ps = psp.tile([128, 512], F32, tag="misc", name="ps_proj")
                for k in range(8):
                    nc.tensor.matmul(
                        ps[:],
                        comb[k][:, m * 128:(m + 1) * 128],
                        comb[k][:, 768 + j * 512:768 + (j + 1) * 512],
                        start=(k == 0), stop=(k == 7),
                    )
                cs = slice(j * 512, (j + 1) * 512)
                if m < 2:
                    nc.vector.tensor_copy(q_t[2 * m][0:64, cs], ps[0:64, :])
                    nc.vector.tensor_copy(q_t[2 * m + 1][0:64, cs], ps[64:128, :])
                else:
                    he, ho = 2 * (m - 2), 2 * (m - 2) + 1
                    nc.vector.tensor_copy(k_t[he][:, cs], ps[:])
                    nc.vector.tensor_copy(k_t[ho][0:64, cs], ps[64:128, :])
                    nc.vector.tensor_copy(k_t[ho][64:128, cs], ps[0:64, :])

            def v_tile(kt):
                ps = psp.tile([128, 256], F32, tag="misc", name="ps_v")
                for k in range(8):
                    nc.tensor.matmul(
                        ps[:],
                        comb[k][:, 768 + kt * 128:768 + (kt + 1) * 128],
                        comb[k][:, 512:768],
                        start=(k == 0), stop=(k == 7),
                    )
                vt = vp_t[kt]
                v_view = vt[:].rearrange("p (h c) -> p h c", c=65)
                ps_view = ps[:].rearrange("p (h c) -> p h c", c=64)
                nc.vector.tensor_copy(v_view[:, :, 0:64], ps_view[:])
                nc.scalar.copy(
                    v_view[:, :, 64:65],
                    ones_t[:, 0:HPC].rearrange("p (h c) -> p h c", c=1),
                )

            pending_norm = []

            def flush_norm():
                while pending_norm:
                    pending_norm.pop(0)()

            def attn_pair(j, hp):
                """AV chains for head pair (2hp, 2hp+1) at chunk j; queues a
                deferred normalization closure so its recip/bcast latency
                hides behind the next pair's matmuls."""
                avs = []
                # denominator rows parked at partitions 0 and 32 so the
                # broadcast matmul rhs has a legal base partition
                den = nwp.tile([33, 512], F16, tag="den", name="den")
                for hh in range(2):
                    h = 2 * hp + hh
                    av = psp.tile([65, 512], F32, tag="av", name="av", bufs=4)
                    nkt = 4 * j + 4

                    def score(kt):
                        """QK matmul + exp for one k-tile. Diagonal-crossing
                        tiles (kt >= 4j) are column-restricted to their
                        causally nonzero range [d4*128, 512); only the first
                        128 columns of that range are triangular and get the
                        mask multiply. Returns AV operands as
                        (expS_slice, out_col_offset, width)."""
                        d4 = kt - 4 * j
                        if d4 < 0:
                            c0, w = 0, 512
                        else:
                            c0, w = d4 * 128, 512 - d4 * 128
                        sp = psp.tile([128, w], F32, tag="mm", name="sp")
                        nc.tensor.matmul(
                            sp[:],
                            k_t[h][:, kt * 128:(kt + 1) * 128],
                            q_t[h][:, j * 512 + c0:(j + 1) * 512],
                            start=True, stop=True,
                        )
                        et = wp.tile([128, w], F16, tag="e", name="et")
                        nc.scalar.activation(et[:], sp[:], AF.Exp, scale=0.125)
                        if d4 < 0:
                            return [(et[:], 0, 512)]
                        emt = wp.tile([128, 128], F16, tag="em", name="emt")
                        nc.vector.tensor_mul(emt[:], et[:, 0:128], consts_t[:])
                        parts = [(emt[:], c0, 128)]
                        if w > 128:
                            parts.append((et[:, 128:w], c0 + 128, w - 128))
                        return parts

                    # stagger: QK(kt+1) issues before AV(kt) so AV's wait on
                    # the fresh expS tile is already satisfied at queue head
                    # and the next LDWEIGHTS can pull ahead.
                    srcs = {0: score(0)}
                    for kt in range(nkt):
                        if kt + 1 < nkt:
                            srcs[kt + 1] = score(kt + 1)
                        parts = srcs.pop(kt)
                        for pi, (src, c0, w) in enumerate(parts):
                            nc.tensor.matmul(
                                av[:, c0:c0 + w],
                                vp_t[kt][:, h * 65:(h + 1) * 65],
                                src,
                                start=(kt == 0),
                                stop=(kt == nkt - 1 and pi == len(parts) - 1),
                                skip_group_check=True,
                            )
                    nc.vector.tensor_copy(den[32 * hh:32 * hh + 1, :], av[64:65, :])
                    avs.append(av)

                def norm():
                    # 1/d computed on ACT as exp(-ln d): both functions live
                    # in the natural_log_exp_and_others table set, so no
                    # table reload against the softmax exps.
                    ln_t = nwp.tile([33, 512], F32, tag="ln", name="ln_t")
                    nc.scalar.activation(ln_t[:], den[:], AF.Ln)
                    rec = nwp.tile([33, 512], F16, tag="rec", name="rec")
                    with nc.allow_low_precision(reason="softmax recip"):
                        nc.scalar.activation(rec[:], ln_t[:], AF.Exp, scale=-1.0)
                    for hh in range(2):
                        h = 2 * hp + hh
                        bc = psp.tile([64, 512], F32, tag="misc", name="bc")
                        nc.tensor.matmul(bc[:], ones_t[32 * hh:32 * hh + 1, 0:64],
                                         rec[32 * hh:32 * hh + 1, :],
                                         start=True, stop=True)
                        bcs = nwp.tile([64, 512], F32, tag="bcs", name="bcs")
                        nc.vector.tensor_copy(bcs[:], bc[:])
                        arow = (h % 2) * 64
                        with nc.allow_low_precision(reason="normalized attn"):
                            nc.vector.tensor_mul(
                                at_t[h // 2][arow:arow + 64, j * 512:(j + 1) * 512],
                                avs[hh][0:64, :], bcs[:],
                            )

                pending_norm.append(norm)

            def wo_chunk(j, on_act=False):
                # out rows for q-chunk j; needs attnT[:, j-chunk] (both pairs
                # of chunk j normalized). The last chunk runs its PSUM copies
                # on ACT, which is idle in the kernel tail.
                for t in range(4 * j, 4 * j + 4):
                    os = nwp.tile([128, D], F16, tag="os", name="os")
                    for n in range(2):
                        wpb = psp.tile([128, 512], F32, tag="mm", name="wpb")
                        for c in range(2):
                            nc.tensor.matmul(
                                wpb[:],
                                at_t[c][:, t * 128:(t + 1) * 128],
                                wo_t[c][:, n * 512:(n + 1) * 512],
                                start=(c == 0), stop=(c == 1),
                            )
                        if on_act:
                            nc.scalar.copy(os[:, n * 512:(n + 1) * 512], wpb[:])
                        else:
                            nc.vector.tensor_copy(os[:, n * 512:(n + 1) * 512], wpb[:])
                    for d4 in range(4):
                        ds = slice(d4 * 256, (d4 + 1) * 256)
                        nc.sync.dma_start(out[t * 128:(t + 1) * 128, ds], os[:, ds])

            for j in range(NQC):
                # pair 0 of chunk j only needs proj groups m=0 (Q heads 0,1)
                # and m=2 (K heads 0,1) plus this chunk's V tiles
                proj_group(j, 0)
                proj_group(j, 2)
                for kt in range(4 * j, 4 * j + 4):
                    v_tile(kt)
                attn_pair(j, 0)
                while len(pending_norm) > 1:
                    pending_norm.pop(0)()
                if 0 < j < NQC - 1:
                    wo_chunk(j - 1)
                proj_group(j, 1)
                proj_group(j, 3)
                attn_pair(j, 1)
                while len(pending_norm) > 1:
                    pending_norm.pop(0)()
            # final sequence: wo(2)'s matmuls keep the PE busy (and HAM warm)
            # while the last pair's normalization chain runs on ACT/DVE
            wo_chunk(NQC - 2)
            flush_norm()
            wo_chunk(NQC - 1, on_act=True)
    return nc


def _make_masks():
    p = np.arange(128)[:, None]
    f = np.arange(128)[None, :]
    return (p <= f).astype(np.float16)


_NC_CACHE = {}


def make_in_maps(x, W_qkv, W_o):
    x = np.ascontiguousarray(np.asarray(x, dtype=np.float32))
    W_qkv = np.ascontiguousarray(np.asarray(W_qkv, dtype=np.float32))
    W_o = np.ascontiguousarray(np.asarray(W_o, dtype=np.float32))
    W_q, W_k, W_v = W_qkv[:, :D], W_qkv[:, D:2 * D], W_qkv[:, 2 * D:]
    masks = _make_masks()

    in_maps = []
    for c in range(N_CORES):
        b, g = c // 4, c % 4
        cols = slice(g * HL, (g + 1) * HL)
        cxv = np.concatenate(
            [W_q[:, cols], W_k[:, cols], W_v[:, cols], x[b].T], axis=1
        ).astype(np.float16)
        in_maps.append({
            "cx": np.ascontiguousarray(cxv),
            "wo": np.ascontiguousarray(W_o[g * HL:(g + 1) * HL, :]),
            "consts": masks,
        })
    return in_maps


def kernel(x, W_qkv, W_o):
    if "nc" not in _NC_CACHE:
        _NC_CACHE["nc"] = build_nc()
    nc = _NC_CACHE["nc"]

    in_maps = make_in_maps(x, W_qkv, W_o)
    res = run_bass_kernel_spmd(nc, in_maps, list(range(N_CORES)))
    out = np.zeros((B, T, D), dtype=np.float32)
    for c in range(N_CORES):
        out[c // 4] += res.results[c]["out"].astype(np.float32)
    return out


# revision 27
# speedup vs baseline: 1.0037x; 1.0037x over previous
"""Multi-head causal attention (B=2, T=2048, D=1024, H=16) on 8 Trainium2
NeuronCores.

Sharding: batch x head-group data/tensor parallel. Core c handles batch
c//4 and heads (c%4)*4 .. +4: W_qkv is split column-wise per head group,
W_o row-wise; each core computes attention for its local heads and a
partial output projection. The host sums the 4 partials per batch
(row-parallel W_o reduction) and stacks the two batches.

Per-core device kernel (fp16 data path, fp32 PSUM accumulate):
  Software-pipelined over q-chunks j=0..3; for each j:
    - projection slice: qkT[:, j*512:+512] = Wqk.T @ xT (per-head Q tiles
      with zeroed partition rows 64-127 and K tiles with finite partner
      rows, so the QK matmul runs with a full K=128 contraction);
    - V k-tiles 4j..4j+3 in natural layout with a per-head ones column
      (the ones column makes the AV matmul also emit the softmax
      denominator row);
    - attention for all 4 heads at chunk j: S.T = KT.T @ QT (PE) ->
      exp(s/8) (ACT, PSUM->SBUF fp16) -> causal-mask multiply on
      diagonal-crossing tiles (DVE) -> AV accumulate [65,512] (PE).
      Normalization is deferred one head pair: denominator rows are
      copied to partitions 0/32 of a [33,512] tile, 1/d computed on ACT
      as exp(-ln d) (same table set as the softmax exps), broadcast via a
      K=1 outer-product matmul (PE), multiplied into attnT (fp32r).
  The W_o projection (fp32r) for chunk j is interleaved into section j+1,
  streaming partial_out rows as fp16.

Softmax skips the max-subtraction: scores are ~N(0,1) after the 1/8 scale,
so exp never overflows fp32 and matches jax.nn.softmax to ~1e-6.
"""
import sys

for _p in ("/opt/trn_rl_repo", "/root/.axon_site/_ro/trn_rl_repo"):
    if _p not in sys.path:
        sys.path.insert(0, _p)

import numpy as np
import concourse.bass as bass
import concourse.mybir as mybir
import concourse.tile as tile
from concourse.vector_clock import ScopedClock
from concourse.bass_utils import run_bass_kernel_spmd

F32 = mybir.dt.float32
F32R = mybir.dt.float32r
F16 = mybir.dt.float16
AF = mybir.ActivationFunctionType

B, T, D = 2, 2048, 1024
N_CORES = 8
HPC = 4            # heads per core
HL = HPC * 64      # 256 local head dims
NKT = T // 128     # 16 k-tiles per head
NQC = T // 512     # 4 q-chunks


class FixedTileContext(tile.TileContext):
    """Works around this walrus build's 1-sync-wait-per-instruction limit.

    1. `_add_instruction`: peel extra waits off any instruction onto
       standalone single-wait nops emitted just before it on the same
       engine (the sequencer executes them in order).
    2. `_drain_and_barrier`: replace the tail drain (which carries one wait
       per outstanding proc) with chained single-wait sync-engine nops
       followed by a wait-free drain.
    """

    def _add_instruction(self, inst):
        si = inst.sync_info
        if si is not None:
            waits = list(si.on_wait)
            if len(waits) > 1:
                eng = getattr(inst, "engine", None)
                eng_obj = self.nc.engines.get(eng) if eng is not None else None
                if eng_obj is not None:
                    for w in waits[:-1]:
                        nop = eng_obj.nop()
                        nop.ins.sync_info = mybir.SyncInfo(on_wait=[w], on_update=[])
                    inst.sync_info = mybir.SyncInfo(
                        on_wait=[waits[-1]], on_update=list(si.on_update)
                    )
        super()._add_instruction(inst)

    def _drain_and_barrier(self, tick_clock, wait_clock):
        vec = tick_clock.global_clock
        for proc in range(len(vec)):
            t = vec[proc]
            if t <= 0:
                continue
            partial = ScopedClock()
            partial.require_at_least(None, proc, t)
            w = self.nc.sync.nop()
            wait_clock.add_sem_waits(w.ins, partial)
        self.nc.sync.drain()
        self.nc.all_engine_barrier()
        assert self.sems is not None
        popped = self.nc._tile_sem_poison_stack.pop()
        assert popped is self._sem_poison
        self.nc.clear_and_free_semaphores(list(self.sems.allocated().values()))
        self.nc.all_engine_barrier()


def build_nc():
    nc = bass.Bass()
    cx = nc.declare_dram_parameter("cx", [D, 2816], F16, isOutput=False)
    wo = nc.declare_dram_parameter("wo", [HL, D], F32R, isOutput=False)
    consts = nc.declare_dram_parameter("consts", [128, 128], F16, isOutput=False)
    out = nc.declare_dram_parameter("out", [T, D], F16, isOutput=True)

    with FixedTileContext(nc) as tc:
        with tc.tile_pool(name="persist", bufs=1) as pp, \
             tc.tile_pool(name="work", bufs=8) as wp, \
             tc.tile_pool(name="nwork", bufs=4) as nwp, \
             tc.tile_pool(name="psum", bufs=2, space="PSUM") as psp:
            consts_t = pp.tile([128, 128], F16, tag="consts")
            nc.sync.dma_start(consts_t[:], consts[:])
            ones_t = pp.tile([128, 64], F16, tag="ones")
            nc.gpsimd.memset(ones_t[:], 1.0)
            wo_t = []
            for c in range(2):
                w = pp.tile([128, D], F32R, tag=f"wo{c}", name=f"wo{c}")
                nc.sync.dma_start(w[:], wo[c * 128:(c + 1) * 128, :])
                wo_t.append(w)

            # comb layout [Wqk 512 | Wv 256 | xT 2048]; DMAs split so the
            # weights and the first token chunk land before later chunks
            comb = []
            bounds = [0, 768, 1280, 1792, 2304, 2816]
            for k in range(8):
                ct = pp.tile([128, 2816], F16, tag=f"comb{k}", name=f"comb{k}")
                for b0, b1 in zip(bounds, bounds[1:]):
                    nc.sync.dma_start(ct[:, b0:b1], cx[k * 128:(k + 1) * 128, b0:b1])
                comb.append(ct)

            # PE warm-up: dummy matmuls on the (tiny, first-to-arrive)
            # consts tile fill the initial DMA wait so the HAM clock gate is
            # already at full rate when the first projection tiles land.
            wu = psp.tile([128, 128], F32, tag="mm", name="wu")
            for _ in range(48):
                nc.tensor.matmul(wu[:], consts_t[:], consts_t[:],
                                 start=True, stop=True)

            # per-head Q and K tiles [128, T]; Q rows 64-127 zeroed
            q_t, k_t = [], []
            for h in range(HPC):
                qt = pp.tile([128, T], F16, tag=f"q{h}", name=f"q{h}")
                nc.gpsimd.memset(qt[64:128, :], 0.0)
                q_t.append(qt)
                kt = pp.tile([128, T], F16, tag=f"k{h}", name=f"k{h}")
                k_t.append(kt)
            vp_t = [pp.tile([128, HPC * 65], F16, tag=f"v{i}", name=f"v{i}")
                    for i in range(NKT)]
            at_t = [pp.tile([128, T], F32R, tag=f"at{c}", name=f"at{c}")
                    for c in range(2)]

            def proj_group(j, m):
                # qkT[:, j-chunk]: m=0,1 -> Q heads (2m, 2m+1); m=2,3 -> K
                ps = psp.tile([128, 512], F32, tag="misc", name="ps_proj")
                for k in range(8):
                    nc.tensor.matmul(
                        ps[:],
                        comb[k][:, m * 128:(m + 1) * 128],
                        comb[k][:, 768 + j * 512:768 + (j + 1) * 512],
                        start=(k == 0), stop=(k == 7),
                    )
                cs = slice(j * 512, (j + 1) * 512)
                if m < 2:
                    nc.vector.tensor_copy(q_t[2 * m][0:64, cs], ps[0:64, :])
                    nc.vector.tensor_copy(q_t[2 * m + 1][0:64, cs], ps[64:128, :])
                else:
                    he, ho = 2 * (m - 2), 2 * (m - 2) + 1
                    nc.vector.tensor_copy(k_t[he][:, cs], ps[:])
                    nc.vector.tensor_copy(k_t[ho][0:64, cs], ps[64:128, :])
                    nc.vector.tensor_copy(k_t[ho][64:128, cs], ps[0:64, :])

            def v_tile(kt):
                ps = psp.tile([128, 256], F32, tag="misc", name="ps_v")
                for k in range(8):
                    nc.tensor.matmul(
                        ps[:],
                        comb[k][:, 768 + kt * 128:768 + (kt + 1) * 128],
                        comb[k][:, 512:768],
                        start=(k == 0), stop=(k == 7),
                    )
                vt = vp_t[kt]
                v_view = vt[:].rearrange("p (h c) -> p h c", c=65)
                ps_view = ps[:].rearrange("p (h c) -> p h c", c=64)
                nc.vector.tensor_copy(v_view[:, :, 0:64], ps_view[:])
                nc.scalar.copy(
                    v_view[:, :, 64:65],
                    ones_t[:, 0:HPC].rearrange("p (h c) -> p h c", c=1),
                )

            pending_norm = []

            def flush_norm():
                while pending_norm:
                    pending_norm.pop(0)()

            def attn_pair(j, hp):
                """AV chains for head pair (2hp, 2hp+1) at chunk j; queues a
                deferred normalization closure so its recip/bcast latency
                hides behind the next pair's matmuls."""
                avs = []
                # denominator rows parked at partitions 0 and 32 so the
                # broadcast matmul rhs has a legal base partition
                den = nwp.tile([33, 512], F16, tag="den", name="den")
                for hh in range(2):
                    h = 2 * hp + hh
                    av = psp.tile([65, 512], F32, tag="av", name="av", bufs=4)
                    nkt = 4 * j + 4

                    def score(kt):
                        """QK matmul + exp for one k-tile. Diagonal-crossing
                        tiles (kt >= 4j) are column-restricted to their
                        causally nonzero range [d4*128, 512); only the first
                        128 columns of that range are triangular and get the
                        mask multiply. Returns AV operands as
                        (expS_slice, out_col_offset, width)."""
                        d4 = kt - 4 * j
                        if d4 < 0:
                            c0, w = 0, 512
                        else:
                            c0, w = d4 * 128, 512 - d4 * 128
                        sp = psp.tile([128, w], F32, tag="mm", name="sp")
                        nc.tensor.matmul(
                            sp[:],
                            k_t[h][:, kt * 128:(kt + 1) * 128],
                            q_t[h][:, j * 512 + c0:(j + 1) * 512],
                            start=True, stop=True,
                        )
                        et = wp.tile([128, w], F16, tag="e", name="et")
                        nc.scalar.activation(et[:], sp[:], AF.Exp, scale=0.125)
                        if d4 < 0:
                            return [(et[:], 0, 512)]
                        emt = wp.tile([128, 128], F16, tag="em", name="emt")
                        nc.vector.tensor_mul(emt[:], et[:, 0:128], consts_t[:])
                        parts = [(emt[:], c0, 128)]
                        if w > 128:
                            parts.append((et[:, 128:w], c0 + 128, w - 128))
                        return parts

                    # stagger: QK(kt+1) issues before AV(kt) so AV's wait on
                    # the fresh expS tile is already satisfied at queue head
                    # and the next LDWEIGHTS can pull ahead.
                    srcs = {0: score(0)}
                    for kt in range(nkt):
                        if kt + 1 < nkt:
                            srcs[kt + 1] = score(kt + 1)
                        parts = srcs.pop(kt)
                        for pi, (src, c0, w) in enumerate(parts):
                            nc.tensor.matmul(
                                av[:, c0:c0 + w],
                                vp_t[kt][:, h * 65:(h + 1) * 65],
                                src,
                                start=(kt == 0),
                                stop=(kt == nkt - 1 and pi == len(parts) - 1),
                                skip_group_check=True,
                            )
                    nc.vector.tensor_copy(den[32 * hh:32 * hh + 1, :], av[64:65, :])
                    avs.append(av)

                def norm():
                    # 1/d computed on ACT as exp(-ln d): both functions live
                    # in the natural_log_exp_and_others table set, so no
                    # table reload against the softmax exps.
                    ln_t = nwp.tile([33, 512], F32, tag="ln", name="ln_t")
                    nc.scalar.activation(ln_t[:], den[:], AF.Ln)
                    rec = nwp.tile([33, 512], F16, tag="rec", name="rec")
                    with nc.allow_low_precision(reason="softmax recip"):
                        nc.scalar.activation(rec[:], ln_t[:], AF.Exp, scale=-1.0)
                    for hh in range(2):
                        h = 2 * hp + hh
                        bc = psp.tile([64, 512], F32, tag="misc", name="bc")
                        nc.tensor.matmul(bc[:], ones_t[32 * hh:32 * hh + 1, 0:64],
                                         rec[32 * hh:32 * hh + 1, :],
                                         start=True, stop=True)
                        bcs = nwp.tile([64, 512], F32, tag="bcs", name="bcs")
                        nc.vector.tensor_copy(bcs[:], bc[:])
                        arow = (h % 2) * 64
                        with nc.allow_low_precision(reason="normalized attn"):
                            nc.vector.tensor_mul(
                                at_t[h // 2][arow:arow + 64, j * 512:(j + 1) * 512],
                                avs[hh][0:64, :], bcs[:],
                            )

                pending_norm.append(norm)

            def wo_chunk(j, on_act=False):
                # out rows for q-chunk j; needs attnT[:, j-chunk] (both pairs
                # of chunk j normalized). The last chunk runs its PSUM copies
                # on ACT, which is idle in the kernel tail.
                for t in range(4 * j, 4 * j + 4):
                    os = nwp.tile([128, D], F16, tag="os", name="os")
                    for n in range(2):
                        wpb = psp.tile([128, 512], F32, tag="mm", name="wpb")
                        for c in range(2):
                            nc.tensor.matmul(
                                wpb[:],
                                at_t[c][:, t * 128:(t + 1) * 128],
                                wo_t[c][:, n * 512:(n + 1) * 512],
                                start=(c == 0), stop=(c == 1),
                            )
                        if on_act:
                            nc.scalar.copy(os[:, n * 512:(n + 1) * 512], wpb[:])
                        else:
                            nc.vector.tensor_copy(os[:, n * 512:(n + 1) * 512], wpb[:])
                    for d4 in range(4):
                        ds = slice(d4 * 256, (d4 + 1) * 256)
                        nc.sync.dma_start(out[t * 128:(t + 1) * 128, ds], os[:, ds])

            for j in range(NQC):
                # pair 0 of chunk j only needs proj groups m=0 (Q heads 0,1)
                # and m=2 (K heads 0,1) plus this chunk's V tiles
                proj_group(j, 0)
                proj_group(j, 2)
                for kt in range(4 * j, 4 * j + 4):
                    v_tile(kt)
                attn_pair(j, 0)
                while len(pending_norm) > 1:
                    pending_norm.pop(0)()
                if 0 < j < NQC - 1:
                    wo_chunk(j - 1)
                proj_group(j, 1)
                proj_group(j, 3)
                attn_pair(j, 1)
                while len(pending_norm) > 1:
                    pending_norm.pop(0)()
            # final sequence: wo(2)'s matmuls keep the PE busy (and HAM warm)
            # while the last pair's normalization chain runs on ACT/DVE
            wo_chunk(NQC - 2)
            flush_norm()
            wo_chunk(NQC - 1, on_act=True)
    return nc


def _make_masks():
    p = np.arange(128)[:, None]
    f = np.arange(128)[None, :]
    return (p <= f).astype(np.float16)


_NC_CACHE = {}


def make_in_maps(x, W_qkv, W_o):
    x = np.ascontiguousarray(np.asarray(x, dtype=np.float32))
    W_qkv = np.ascontiguousarray(np.asarray(W_qkv, dtype=np.float32))
    W_o = np.ascontiguousarray(np.asarray(W_o, dtype=np.float32))
    W_q, W_k, W_v = W_qkv[:, :D], W_qkv[:, D:2 * D], W_qkv[:, 2 * D:]
    masks = _make_masks()

    in_maps = []
    for c in range(N_CORES):
        b, g = c // 4, c % 4
        cols = slice(g * HL, (g + 1) * HL)
        cxv = np.concatenate(
            [W_q[:, cols], W_k[:, cols], W_v[:, cols], x[b].T], axis=1
        ).astype(np.float16)
        in_maps.append({
            "cx": np.ascontiguousarray(cxv),
            "wo": np.ascontiguousarray(W_o[g * HL:(g + 1) * HL, :]),
            "consts": masks,
        })
    return in_maps


def kernel(x, W_qkv, W_o):
    if "nc" not in _NC_CACHE:
        _NC_CACHE["nc"] = build_nc()
    nc = _NC_CACHE["nc"]

    in_maps = make_in_maps(x, W_qkv, W_o)
    res = run_bass_kernel_spmd(nc, in_maps, list(range(N_CORES)))
    out = np.zeros((B, T, D), dtype=np.float32)
    for c in range(N_CORES):
        out[c // 4] += res.results[c]["out"].astype(np.float32)
    return out


# revision 28
# speedup vs baseline: 1.0038x; 1.0000x over previous
"""Multi-head causal attention (B=2, T=2048, D=1024, H=16) on 8 Trainium2
NeuronCores.

Sharding: batch x head-group data/tensor parallel. Core c handles batch
c//4 and heads (c%4)*4 .. +4: W_qkv is split column-wise per head group,
W_o row-wise; each core computes attention for its local heads and a
partial output projection. The host sums the 4 partials per batch
(row-parallel W_o reduction) and stacks the two batches.

Per-core device kernel (fp16 data path, fp32 PSUM accumulate):
  Software-pipelined over q-chunks j=0..3; for each j:
    - projection slice: qkT[:, j*512:+512] = Wqk.T @ xT (per-head Q tiles
      with zeroed partition rows 64-127 and K tiles with finite partner
      rows, so the QK matmul runs with a full K=128 contraction);
    - V k-tiles 4j..4j+3 in natural layout with a per-head ones column
      (the ones column makes the AV matmul also emit the softmax
      denominator row);
    - attention for all 4 heads at chunk j: S.T = KT.T @ QT (PE) ->
      exp(s/8) (ACT, PSUM->SBUF fp16) -> causal-mask multiply on
      diagonal-crossing tiles (DVE) -> AV accumulate [65,512] (PE).
      Normalization is deferred one head pair: denominator rows are
      copied to partitions 0/32 of a [33,512] tile, 1/d computed on ACT
      as exp(-ln d) (same table set as the softmax exps), broadcast via a
      K=1 outer-product matmul (PE), multiplied into attnT (fp32r).
  The W_o projection (fp32r) for chunk j is interleaved into section j+1,
  streaming partial_out rows as fp16.

Softmax skips the max-subtraction: scores are ~N(0,1) after the 1/8 scale,
so exp never overflows fp32 and matches jax.nn.softmax to ~1e-6.
"""
import sys

for _p in ("/opt/trn_rl_repo", "/root/.axon_site/_ro/trn_rl_repo"):
    if _p not in sys.path:
        sys.path.insert(0, _p)

import numpy as np
import concourse.bass as bass
import concourse.mybir as mybir
import concourse.tile as tile
from concourse.vector_clock import ScopedClock
from concourse.bass_utils import run_bass_kernel_spmd

F32 = mybir.dt.float32
F32R = mybir.dt.float32r
F16 = mybir.dt.float16
AF = mybir.ActivationFunctionType

B, T, D = 2, 2048, 1024
N_CORES = 8
HPC = 4            # heads per core
HL = HPC * 64      # 256 local head dims
NKT = T // 128     # 16 k-tiles per head
NQC = T // 512     # 4 q-chunks


class FixedTileContext(tile.TileContext):
    """Works around this walrus build's 1-sync-wait-per-instruction limit.

    1. `_add_instruction`: peel extra waits off any instruction onto
       standalone single-wait nops emitted just before it on the same
       engine (the sequencer executes them in order).
    2. `_drain_and_barrier`: replace the tail drain (which carries one wait
       per outstanding proc) with chained single-wait sync-engine nops
       followed by a wait-free drain.
    """

    def _add_instruction(self, inst):
        si = inst.sync_info
        if si is not None:
            waits = list(si.on_wait)
            if len(waits) > 1:
                eng = getattr(inst, "engine", None)
                eng_obj = self.nc.engines.get(eng) if eng is not None else None
                if eng_obj is not None:
                    for w in waits[:-1]:
                        nop = eng_obj.nop()
                        nop.ins.sync_info = mybir.SyncInfo(on_wait=[w], on_update=[])
                    inst.sync_info = mybir.SyncInfo(
                        on_wait=[waits[-1]], on_update=list(si.on_update)
                    )
        super()._add_instruction(inst)

    def _drain_and_barrier(self, tick_clock, wait_clock):
        vec = tick_clock.global_clock
        for proc in range(len(vec)):
            t = vec[proc]
            if t <= 0:
                continue
            partial = ScopedClock()
            partial.require_at_least(None, proc, t)
            w = self.nc.sync.nop()
            wait_clock.add_sem_waits(w.ins, partial)
        self.nc.sync.drain()
        self.nc.all_engine_barrier()
        assert self.sems is not None
        popped = self.nc._tile_sem_poison_stack.pop()
        assert popped is self._sem_poison
        self.nc.clear_and_free_semaphores(list(self.sems.allocated().values()))
        self.nc.all_engine_barrier()


def build_nc():
    nc = bass.Bass()
    cx = nc.declare_dram_parameter("cx", [D, 2816], F16, isOutput=False)
    wo = nc.declare_dram_parameter("wo", [HL, D], F32R, isOutput=False)
    consts = nc.declare_dram_parameter("consts", [128, 128], F16, isOutput=False)
    out = nc.declare_dram_parameter("out", [T, D], F16, isOutput=True)

    with FixedTileContext(nc) as tc:
        with tc.tile_pool(name="persist", bufs=1) as pp, \
             tc.tile_pool(name="work", bufs=8) as wp, \
             tc.tile_pool(name="nwork", bufs=4) as nwp, \
             tc.tile_pool(name="psum", bufs=2, space="PSUM") as psp:
            consts_t = pp.tile([128, 128], F16, tag="consts")
            nc.sync.dma_start(consts_t[:], consts[:])
            ones_t = pp.tile([128, 64], F16, tag="ones")
            nc.gpsimd.memset(ones_t[:], 1.0)
            wo_t = []
            for c in range(2):
                w = pp.tile([128, D], F32R, tag=f"wo{c}", name=f"wo{c}")
                nc.sync.dma_start(w[:], wo[c * 128:(c + 1) * 128, :])
                wo_t.append(w)

            # comb layout [Wqk 512 | Wv 256 | xT 2048]; DMAs split so the
            # weights and the first token chunk land before later chunks
            comb = []
            bounds = [0, 768, 1280, 1792, 2304, 2816]
            for k in range(8):
                ct = pp.tile([128, 2816], F16, tag=f"comb{k}", name=f"comb{k}")
                for b0, b1 in zip(bounds, bounds[1:]):
                    nc.sync.dma_start(ct[:, b0:b1], cx[k * 128:(k + 1) * 128, b0:b1])
                comb.append(ct)

            # PE warm-up: dummy matmuls on the (tiny, first-to-arrive)
            # consts tile fill the initial DMA wait so the HAM clock gate is
            # already at full rate when the first projection tiles land.
            wu = psp.tile([128, 128], F32, tag="mm", name="wu")
            for _ in range(48):
                nc.tensor.matmul(wu[:], consts_t[:], consts_t[:],
                                 start=True, stop=True)

            # per-head Q and K tiles [128, T]; Q rows 64-127 zeroed
            q_t, k_t = [], []
            for h in range(HPC):
                qt = pp.tile([128, T], F16, tag=f"q{h}", name=f"q{h}")
                nc.gpsimd.memset(qt[64:128, :], 0.0)
                q_t.append(qt)
                kt = pp.tile([128, T], F16, tag=f"k{h}", name=f"k{h}")
                nc.gpsimd.memset(kt[64:128, :], 0.0)
                k_t.append(kt)
            vp_t = [pp.tile([128, HPC * 65], F16, tag=f"v{i}", name=f"v{i}")
                    for i in range(NKT)]
            at_t = [pp.tile([128, T], F32R, tag=f"at{c}", name=f"at{c}")
                    for c in range(2)]

            def proj_group(j, m):
                # qkT[:, j-chunk]: m=0,1 -> Q heads (2m, 2m+1); m=2,3 -> K
                ps = psp.tile([128, 512], F32, tag="misc", name="ps_proj")
                for k in range(8):
                    nc.tensor.matmul(
                        ps[:],
                        comb[k][:, m * 128:(m + 1) * 128],
                        comb[k][:, 768 + j * 512:768 + (j + 1) * 512],
                        start=(k == 0), stop=(k == 7),
                    )
                cs = slice(j * 512, (j + 1) * 512)
                if m < 2:
                    nc.vector.tensor_copy(q_t[2 * m][0:64, cs], ps[0:64, :])
                    nc.vector.tensor_copy(q_t[2 * m + 1][0:64, cs], ps[64:128, :])
                else:
                    he, ho = 2 * (m - 2), 2 * (m - 2) + 1
                    nc.vector.tensor_copy(k_t[he][0:64, cs], ps[0:64, :])
                    nc.vector.tensor_copy(k_t[ho][0:64, cs], ps[64:128, :])

            def v_tile(kt):
                ps = psp.tile([128, 256], F32, tag="misc", name="ps_v")
                for k in range(8):
                    nc.tensor.matmul(
                        ps[:],
                        comb[k][:, 768 + kt * 128:768 + (kt + 1) * 128],
                        comb[k][:, 512:768],
                        start=(k == 0), stop=(k == 7),
                    )
                vt = vp_t[kt]
                v_view = vt[:].rearrange("p (h c) -> p h c", c=65)
                ps_view = ps[:].rearrange("p (h c) -> p h c", c=64)
                nc.vector.tensor_copy(v_view[:, :, 0:64], ps_view[:])
                nc.scalar.copy(
                    v_view[:, :, 64:65],
                    ones_t[:, 0:HPC].rearrange("p (h c) -> p h c", c=1),
                )

            pending_norm = []

            def flush_norm():
                while pending_norm:
                    pending_norm.pop(0)()

            def attn_pair(j, hp):
                """AV chains for head pair (2hp, 2hp+1) at chunk j; queues a
                deferred normalization closure so its recip/bcast latency
                hides behind the next pair's matmuls."""
                avs = []
                # denominator rows parked at partitions 0 and 32 so the
                # broadcast matmul rhs has a legal base partition
                den = nwp.tile([33, 512], F16, tag="den", name="den")
                for hh in range(2):
                    h = 2 * hp + hh
                    av = psp.tile([65, 512], F32, tag="av", name="av", bufs=4)
                    nkt = 4 * j + 4

                    def score(kt):
                        """QK matmul + exp for one k-tile. Diagonal-crossing
                        tiles (kt >= 4j) are column-restricted to their
                        causally nonzero range [d4*128, 512); only the first
                        128 columns of that range are triangular and get the
                        mask multiply. Returns AV operands as
                        (expS_slice, out_col_offset, width)."""
                        d4 = kt - 4 * j
                        if d4 < 0:
                            c0, w = 0, 512
                        else:
                            c0, w = d4 * 128, 512 - d4 * 128
                        sp = psp.tile([128, w], F32, tag="mm", name="sp")
                        nc.tensor.matmul(
                            sp[:],
                            k_t[h][:, kt * 128:(kt + 1) * 128],
                            q_t[h][:, j * 512 + c0:(j + 1) * 512],
                            start=True, stop=True,
                        )
                        et = wp.tile([128, w], F16, tag="e", name="et")
                        nc.scalar.activation(et[:], sp[:], AF.Exp, scale=0.125)
                        if d4 < 0:
                            return [(et[:], 0, 512)]
                        emt = wp.tile([128, 128], F16, tag="em", name="emt")
                        nc.vector.tensor_mul(emt[:], et[:, 0:128], consts_t[:])
                        parts = [(emt[:], c0, 128)]
                        if w > 128:
                            parts.append((et[:, 128:w], c0 + 128, w - 128))
                        return parts

                    # stagger: QK(kt+1) issues before AV(kt) so AV's wait on
                    # the fresh expS tile is already satisfied at queue head
                    # and the next LDWEIGHTS can pull ahead.
                    srcs = {0: score(0)}
                    for kt in range(nkt):
                        if kt + 1 < nkt:
                            srcs[kt + 1] = score(kt + 1)
                        parts = srcs.pop(kt)
                        for pi, (src, c0, w) in enumerate(parts):
                            nc.tensor.matmul(
                                av[:, c0:c0 + w],
                                vp_t[kt][:, h * 65:(h + 1) * 65],
                                src,
                                start=(kt == 0),
                                stop=(kt == nkt - 1 and pi == len(parts) - 1),
                                skip_group_check=True,
                            )
                    nc.vector.tensor_copy(den[32 * hh:32 * hh + 1, :], av[64:65, :])
                    avs.append(av)

                def norm():
                    # 1/d computed on ACT as exp(-ln d): both functions live
                    # in the natural_log_exp_and_others table set, so no
                    # table reload against the softmax exps.
                    ln_t = nwp.tile([33, 512], F32, tag="ln", name="ln_t")
                    nc.scalar.activation(ln_t[:], den[:], AF.Ln)
                    rec = nwp.tile([33, 512], F16, tag="rec", name="rec")
                    with nc.allow_low_precision(reason="softmax recip"):
                        nc.scalar.activation(rec[:], ln_t[:], AF.Exp, scale=-1.0)
                    for hh in range(2):
                        h = 2 * hp + hh
                        bc = psp.tile([64, 512], F32, tag="misc", name="bc")
                        nc.tensor.matmul(bc[:], ones_t[32 * hh:32 * hh + 1, 0:64],
                                         rec[32 * hh:32 * hh + 1, :],
                                         start=True, stop=True)
                        bcs = nwp.tile([64, 512], F32, tag="bcs", name="bcs")
                        nc.vector.tensor_copy(bcs[:], bc[:])
                        arow = (h % 2) * 64
                        with nc.allow_low_precision(reason="normalized attn"):
                            nc.vector.tensor_mul(
                                at_t[h // 2][arow:arow + 64, j * 512:(j + 1) * 512],
                                avs[hh][0:64, :], bcs[:],
                            )

                pending_norm.append(norm)

            def wo_chunk(j, on_act=False):
                # out rows for q-chunk j; needs attnT[:, j-chunk] (both pairs
                # of chunk j normalized). The last chunk runs its PSUM copies
                # on ACT, which is idle in the kernel tail.
                for t in range(4 * j, 4 * j + 4):
                    os = nwp.tile([128, D], F16, tag="os", name="os")
                    for n in range(2):
                        wpb = psp.tile([128, 512], F32, tag="mm", name="wpb")
                        for c in range(2):
                            nc.tensor.matmul(
                                wpb[:],
                                at_t[c][:, t * 128:(t + 1) * 128],
                                wo_t[c][:, n * 512:(n + 1) * 512],
                                start=(c == 0), stop=(c == 1),
                            )
                        if on_act:
                            nc.scalar.copy(os[:, n * 512:(n + 1) * 512], wpb[:])
                        else:
                            nc.vector.tensor_copy(os[:, n * 512:(n + 1) * 512], wpb[:])
                    for d4 in range(4):
                        ds = slice(d4 * 256, (d4 + 1) * 256)
                        nc.sync.dma_start(out[t * 128:(t + 1) * 128, ds], os[:, ds])

            for j in range(NQC):
                # pair 0 of chunk j only needs proj groups m=0 (Q heads 0,1)
                # and m=2 (K heads 0,1) plus this chunk's V tiles
                proj_group(j, 0)
                proj_group(j, 2)
                for kt in range(4 * j, 4 * j + 4):
                    v_tile(kt)
                attn_pair(j, 0)
                while len(pending_norm) > 1:
                    pending_norm.pop(0)()
                if 0 < j < NQC - 1:
                    wo_chunk(j - 1)
                proj_group(j, 1)
                proj_group(j, 3)
                attn_pair(j, 1)
                while len(pending_norm) > 1:
                    pending_norm.pop(0)()
            # final sequence: wo(2)'s matmuls keep the PE busy (and HAM warm)
            # while the last pair's normalization chain runs on ACT/DVE
            wo_chunk(NQC - 2)
            flush_norm()
            wo_chunk(NQC - 1, on_act=True)
    return nc


def _make_masks():
    p = np.arange(128)[:, None]
    f = np.arange(128)[None, :]
    return (p <= f).astype(np.float16)


_NC_CACHE = {}


def make_in_maps(x, W_qkv, W_o):
    x = np.ascontiguousarray(np.asarray(x, dtype=np.float32))
    W_qkv = np.ascontiguousarray(np.asarray(W_qkv, dtype=np.float32))
    W_o = np.ascontiguousarray(np.asarray(W_o, dtype=np.float32))
    W_q, W_k, W_v = W_qkv[:, :D], W_qkv[:, D:2 * D], W_qkv[:, 2 * D:]
    masks = _make_masks()

    in_maps = []
    for c in range(N_CORES):
        b, g = c // 4, c % 4
        cols = slice(g * HL, (g + 1) * HL)
        cxv = np.concatenate(
            [W_q[:, cols], W_k[:, cols], W_v[:, cols], x[b].T], axis=1
        ).astype(np.float16)
        in_maps.append({
            "cx": np.ascontiguousarray(cxv),
            "wo": np.ascontiguousarray(W_o[g * HL:(g + 1) * HL, :]),
            "consts": masks,
        })
    return in_maps


def kernel(x, W_qkv, W_o):
    if "nc" not in _NC_CACHE:
        _NC_CACHE["nc"] = build_nc()
    nc = _NC_CACHE["nc"]

    in_maps = make_in_maps(x, W_qkv, W_o)
    res = run_bass_kernel_spmd(nc, in_maps, list(range(N_CORES)))
    out = np.zeros((B, T, D), dtype=np.float32)
    for c in range(N_CORES):
        out[c // 4] += res.results[c]["out"].astype(np.float32)
    return out


# revision 29
# speedup vs baseline: 1.0525x; 1.0485x over previous
"""Multi-head causal attention (B=2, T=2048, D=1024, H=16) on 8 Trainium2
NeuronCores.

Sharding: batch x head-group data/tensor parallel. Core c handles batch
c//4 and heads (c%4)*4 .. +4: W_qkv is split column-wise per head group,
W_o row-wise; each core computes attention for its local heads and a
partial output projection. The host sums the 4 partials per batch
(row-parallel W_o reduction) and stacks the two batches.

Per-core device kernel (fp16 data path, fp32 PSUM accumulate):
  Software-pipelined over q-chunks j=0..3; for each j:
    - projection slice: qkT[:, j*512:+512] = Wqk.T @ xT (per-head Q tiles
      with zeroed partition rows 64-127 and K tiles with finite partner
      rows, so the QK matmul runs with a full K=128 contraction);
    - V k-tiles 4j..4j+3 in natural layout with a per-head ones column
      (the ones column makes the AV matmul also emit the softmax
      denominator row);
    - attention for all 4 heads at chunk j: S.T = KT.T @ QT (PE) ->
      exp(s/8) (ACT, PSUM->SBUF fp16) -> causal-mask multiply on
      diagonal-crossing tiles (DVE) -> AV accumulate [65,512] (PE).
      Normalization is deferred one head pair: denominator rows are
      copied to partitions 0/32 of a [33,512] tile, 1/d computed on ACT
      as exp(-ln d) (same table set as the softmax exps), broadcast via a
      K=1 outer-product matmul (PE), multiplied into attnT (fp32r).
  The W_o projection (fp32r) for chunk j is interleaved into section j+1,
  streaming partial_out rows as fp16.

Softmax skips the max-subtraction: scores are ~N(0,1) after the 1/8 scale,
so exp never overflows fp32 and matches jax.nn.softmax to ~1e-6.
"""
import sys

for _p in ("/opt/trn_rl_repo", "/root/.axon_site/_ro/trn_rl_repo"):
    if _p not in sys.path:
        sys.path.insert(0, _p)

import numpy as np
import concourse.bass as bass
import concourse.mybir as mybir
import concourse.tile as tile
from concourse.vector_clock import ScopedClock
from concourse.bass_utils import run_bass_kernel_spmd

F32 = mybir.dt.float32
F32R = mybir.dt.float32r
F16 = mybir.dt.float16
AF = mybir.ActivationFunctionType

B, T, D = 2, 2048, 1024
N_CORES = 8
HPC = 4            # heads per core
HL = HPC * 64      # 256 local head dims
NKT = T // 128     # 16 k-tiles per head
NQC = T // 512     # 4 q-chunks


class FixedTileContext(tile.TileContext):
    """Works around this walrus build's 1-sync-wait-per-instruction limit.

    1. `_add_instruction`: peel extra waits off any instruction onto
       standalone single-wait nops emitted just before it on the same
       engine (the sequencer executes them in order).
    2. `_drain_and_barrier`: replace the tail drain (which carries one wait
       per outstanding proc) with chained single-wait sync-engine nops
       followed by a wait-free drain.
    """

    def _add_instruction(self, inst):
        si = inst.sync_info
        if si is not None:
            waits = list(si.on_wait)
            if len(waits) > 1:
                eng = getattr(inst, "engine", None)
                eng_obj = self.nc.engines.get(eng) if eng is not None else None
                if eng_obj is not None:
                    for w in waits[:-1]:
                        nop = eng_obj.nop()
                        nop.ins.sync_info = mybir.SyncInfo(on_wait=[w], on_update=[])
                    inst.sync_info = mybir.SyncInfo(
                        on_wait=[waits[-1]], on_update=list(si.on_update)
                    )
        super()._add_instruction(inst)

    def _drain_and_barrier(self, tick_clock, wait_clock):
        vec = tick_clock.global_clock
        for proc in range(len(vec)):
            t = vec[proc]
            if t <= 0:
                continue
            partial = ScopedClock()
            partial.require_at_least(None, proc, t)
            w = self.nc.sync.nop()
            wait_clock.add_sem_waits(w.ins, partial)
        self.nc.sync.drain()
        self.nc.all_engine_barrier()
        assert self.sems is not None
        popped = self.nc._tile_sem_poison_stack.pop()
        assert popped is self._sem_poison
        self.nc.clear_and_free_semaphores(list(self.sems.allocated().values()))
        self.nc.all_engine_barrier()


def build_nc():
    nc = bass.Bass()
    cx = nc.declare_dram_parameter("cx", [D, 2816], F16, isOutput=False)
    wo = nc.declare_dram_parameter("wo", [HL, D], F32R, isOutput=False)
    consts = nc.declare_dram_parameter("consts", [128, 128], F16, isOutput=False)
    out = nc.declare_dram_parameter("out", [T, D], F16, isOutput=True)

    with FixedTileContext(nc) as tc:
        with tc.tile_pool(name="persist", bufs=1) as pp, \
             tc.tile_pool(name="work", bufs=8) as wp, \
             tc.tile_pool(name="nwork", bufs=4) as nwp, \
             tc.tile_pool(name="psum", bufs=2, space="PSUM") as psp:
            consts_t = pp.tile([128, 128], F16, tag="consts")
            nc.sync.dma_start(consts_t[:], consts[:])
            ones_t = pp.tile([128, 64], F16, tag="ones")
            nc.gpsimd.memset(ones_t[:], 1.0)
            wo_t = []
            for c in range(2):
                w = pp.tile([128, D], F32R, tag=f"wo{c}", name=f"wo{c}")
                nc.sync.dma_start(w[:], wo[c * 128:(c + 1) * 128, :])
                wo_t.append(w)

            # comb layout [Wqk 512 | Wv 256 | xT 2048]; DMAs split so the
            # weights and the first token chunk land before later chunks
            comb = []
            bounds = [0, 768, 1280, 1792, 2304, 2816]
            for k in range(8):
                ct = pp.tile([128, 2816], F16, tag=f"comb{k}", name=f"comb{k}")
                for b0, b1 in zip(bounds, bounds[1:]):
                    nc.sync.dma_start(ct[:, b0:b1], cx[k * 128:(k + 1) * 128, b0:b1])
                comb.append(ct)

            # PE warm-up: dummy matmuls on the (tiny, first-to-arrive)
            # consts tile fill the initial DMA wait so the HAM clock gate is
            # already at full rate when the first projection tiles land.
            wu = psp.tile([128, 128], F32, tag="mm", name="wu")
            for _ in range(48):
                nc.tensor.matmul(wu[:], consts_t[:], consts_t[:],
                                 start=True, stop=True)

            # per-head Q and K tiles [128, T]; Q rows 64-127 zeroed
            q_t, k_t = [], []
            for h in range(HPC):
                qt = pp.tile([128, T], F16, tag=f"q{h}", name=f"q{h}")
                nc.gpsimd.memset(qt[64:128, :], 0.0)
                q_t.append(qt)
                kt = pp.tile([128, T], F16, tag=f"k{h}", name=f"k{h}")
                nc.gpsimd.memset(kt[64:128, :], 0.0)
                k_t.append(kt)
            vp_t = [pp.tile([128, HPC * 65], F16, tag=f"v{i}", name=f"v{i}")
                    for i in range(NKT)]
            at_t = [pp.tile([128, T], F32R, tag=f"at{c}", name=f"at{c}")
                    for c in range(2)]

            def proj_group(j, m):
                # qkT[:, j-chunk]: m=0,1 -> Q heads (2m, 2m+1); m=2,3 -> K
                ps = psp.tile([128, 512], F32, tag="misc", name="ps_proj")
                for k in range(8):
                    nc.tensor.matmul(
                        ps[:],
                        comb[k][:, m * 128:(m + 1) * 128],
                        comb[k][:, 768 + j * 512:768 + (j + 1) * 512],
                        start=(k == 0), stop=(k == 7),
                    )
                cs = slice(j * 512, (j + 1) * 512)
                if m < 2:
                    nc.vector.tensor_copy(q_t[2 * m][0:64, cs], ps[0:64, :])
                    nc.vector.tensor_copy(q_t[2 * m + 1][0:64, cs], ps[64:128, :])
                else:
                    he, ho = 2 * (m - 2), 2 * (m - 2) + 1
                    nc.vector.tensor_copy(k_t[he][0:64, cs], ps[0:64, :])
                    nc.vector.tensor_copy(k_t[ho][0:64, cs], ps[64:128, :])

            def v_tile(kt):
                ps = psp.tile([128, 256], F32, tag="misc", name="ps_v")
                for k in range(8):
                    nc.tensor.matmul(
                        ps[:],
                        comb[k][:, 768 + kt * 128:768 + (kt + 1) * 128],
                        comb[k][:, 512:768],
                        start=(k == 0), stop=(k == 7),
                    )
                vt = vp_t[kt]
                v_view = vt[:].rearrange("p (h c) -> p h c", c=65)
                ps_view = ps[:].rearrange("p (h c) -> p h c", c=64)
                nc.vector.tensor_copy(v_view[:, :, 0:64], ps_view[:])
                nc.scalar.copy(
                    v_view[:, :, 64:65],
                    ones_t[:, 0:HPC].rearrange("p (h c) -> p h c", c=1),
                )

            pending_norm = []

            def flush_norm():
                while pending_norm:
                    pending_norm.pop(0)()

            def attn_pair(j, hp):
                """AV chains for head pair (2hp, 2hp+1) at chunk j; queues a
                deferred normalization closure so its recip/bcast latency
                hides behind the next pair's matmuls."""
                avs = []
                # denominator rows parked at partitions 0 and 32 so the
                # broadcast matmul rhs has a legal base partition
                den = nwp.tile([33, 512], F16, tag="den", name="den")
                for hh in range(2):
                    h = 2 * hp + hh
                    av = psp.tile([65, 512], F32, tag="av", name="av", bufs=4)
                    nkt = 4 * j + 4

                    def score(kt):
                        """QK matmul + exp for one k-tile. Diagonal-crossing
                        tiles (kt >= 4j) are column-restricted to their
                        causally nonzero range [d4*128, 512); only the first
                        128 columns of that range are triangular and get the
                        mask multiply. Returns AV operands as
                        (expS_slice, out_col_offset, width)."""
                        d4 = kt - 4 * j
                        if d4 < 0:
                            c0, w = 0, 512
                        else:
                            c0, w = d4 * 128, 512 - d4 * 128
                        sp = psp.tile([128, w], F32, tag="mm", name="sp")
                        nc.tensor.matmul(
                            sp[:],
                            k_t[h][:, kt * 128:(kt + 1) * 128],
                            q_t[h][:, j * 512 + c0:(j + 1) * 512],
                            start=True, stop=True,
                        )
                        et = wp.tile([128, w], F16, tag="e", name="et")
                        nc.scalar.activation(et[:], sp[:], AF.Exp, scale=0.125)
                        if d4 < 0:
                            return [(et[:], 0, 512)]
                        emt = wp.tile([128, 128], F16, tag="em", name="emt")
                        nc.vector.tensor_mul(emt[:], et[:, 0:128], consts_t[:])
                        parts = [(emt[:], c0, 128)]
                        if w > 128:
                            parts.append((et[:, 128:w], c0 + 128, w - 128))
                        return parts

                    # stagger: QK(kt+1) issues before AV(kt) so AV's wait on
                    # the fresh expS tile is already satisfied at queue head
                    # and the next LDWEIGHTS can pull ahead.
                    srcs = {0: score(0)}
                    for kt in range(nkt):
                        if kt + 1 < nkt:
                            srcs[kt + 1] = score(kt + 1)
                        parts = srcs.pop(kt)
                        for pi, (src, c0, w) in enumerate(parts):
                            nc.tensor.matmul(
                                av[:, c0:c0 + w],
                                vp_t[kt][:, h * 65:(h + 1) * 65],
                                src,
                                start=(kt == 0),
                                stop=(kt == nkt - 1 and pi == len(parts) - 1),
                                skip_group_check=True,
                            )
                    nc.vector.tensor_copy(den[32 * hh:32 * hh + 1, :], av[64:65, :])
                    avs.append(av)

                def norm():
                    # 1/d computed on ACT as exp(-ln d): both functions live
                    # in the natural_log_exp_and_others table set, so no
                    # table reload against the softmax exps.
                    ln_t = nwp.tile([33, 512], F32, tag="ln", name="ln_t")
                    nc.scalar.activation(ln_t[:], den[:], AF.Ln)
                    rec = nwp.tile([33, 512], F16, tag="rec", name="rec")
                    with nc.allow_low_precision(reason="softmax recip"):
                        nc.scalar.activation(rec[:], ln_t[:], AF.Exp, scale=-1.0)
                    for hh in range(2):
                        h = 2 * hp + hh
                        bc = psp.tile([64, 512], F32, tag="misc", name="bc")
                        nc.tensor.matmul(bc[:], ones_t[32 * hh:32 * hh + 1, 0:64],
                                         rec[32 * hh:32 * hh + 1, :],
                                         start=True, stop=True)
                        bcs = nwp.tile([64, 512], F32, tag="bcs", name="bcs")
                        nc.vector.tensor_copy(bcs[:], bc[:])
                        arow = (h % 2) * 64
                        with nc.allow_low_precision(reason="normalized attn"):
                            nc.vector.tensor_mul(
                                at_t[h // 2][arow:arow + 64, j * 512:(j + 1) * 512],
                                avs[hh][0:64, :], bcs[:],
                            )

                pending_norm.append(norm)

            def wo_chunk(j, on_act=False):
                # out rows for q-chunk j; needs attnT[:, j-chunk] (both pairs
                # of chunk j normalized). The last chunk runs its PSUM copies
                # on ACT, which is idle in the kernel tail.
                for t in range(4 * j, 4 * j + 4):
                    os = nwp.tile([128, D], F16, tag="os", name="os")
                    for n in range(2):
                        wpb = psp.tile([128, 512], F32, tag="mm", name="wpb")
                        for c in range(2):
                            nc.tensor.matmul(
                                wpb[:],
                                at_t[c][:, t * 128:(t + 1) * 128],
                                wo_t[c][:, n * 512:(n + 1) * 512],
                                start=(c == 0), stop=(c == 1),
                            )
                        if on_act:
                            nc.scalar.copy(os[:, n * 512:(n + 1) * 512], wpb[:])
                        else:
                            nc.vector.tensor_copy(os[:, n * 512:(n + 1) * 512], wpb[:])
                    for d2 in range(2):
                        ds = slice(d2 * 512, (d2 + 1) * 512)
                        # final chunk: split DMA issue between the SP and ACT
                        # sequencers (ACT is idle in the tail; each dma_start
                        # costs ~600 ns of sequencer issue time)
                        eng = nc.scalar if (on_act and d2 == 1) else nc.sync
                        eng.dma_start(out[t * 128:(t + 1) * 128, ds], os[:, ds])

            for j in range(NQC):
                # pair 0 of chunk j only needs proj groups m=0 (Q heads 0,1)
                # and m=2 (K heads 0,1) plus this chunk's V tiles
                proj_group(j, 0)
                proj_group(j, 2)
                for kt in range(4 * j, 4 * j + 4):
                    v_tile(kt)
                attn_pair(j, 0)
                while len(pending_norm) > 1:
                    pending_norm.pop(0)()
                if 0 < j < NQC - 1:
                    wo_chunk(j - 1)
                proj_group(j, 1)
                proj_group(j, 3)
                attn_pair(j, 1)
                while len(pending_norm) > 1:
                    pending_norm.pop(0)()
            # final sequence: wo(2)'s matmuls keep the PE busy (and HAM warm)
            # while the last pair's normalization chain runs on ACT/DVE
            wo_chunk(NQC - 2)
            flush_norm()
            wo_chunk(NQC - 1, on_act=True)
    return nc


def _make_masks():
    p = np.arange(128)[:, None]
    f = np.arange(128)[None, :]
    return (p <= f).astype(np.float16)


_NC_CACHE = {}


def make_in_maps(x, W_qkv, W_o):
    x = np.ascontiguousarray(np.asarray(x, dtype=np.float32))
    W_qkv = np.ascontiguousarray(np.asarray(W_qkv, dtype=np.float32))
    W_o = np.ascontiguousarray(np.asarray(W_o, dtype=np.float32))
    W_q, W_k, W_v = W_qkv[:, :D], W_qkv[:, D:2 * D], W_qkv[:, 2 * D:]
    masks = _make_masks()

    in_maps = []
    for c in range(N_CORES):
        b, g = c // 4, c % 4
        cols = slice(g * HL, (g + 1) * HL)
        cxv = np.concatenate(
            [W_q[:, cols], W_k[:, cols], W_v[:, cols], x[b].T], axis=1
        ).astype(np.float16)
        in_maps.append({
            "cx": np.ascontiguousarray(cxv),
            "wo": np.ascontiguousarray(W_o[g * HL:(g + 1) * HL, :]),
            "consts": masks,
        })
    return in_maps


def kernel(x, W_qkv, W_o):
    if "nc" not in _NC_CACHE:
        _NC_CACHE["nc"] = build_nc()
    nc = _NC_CACHE["nc"]

    in_maps = make_in_maps(x, W_qkv, W_o)
    res = run_bass_kernel_spmd(nc, in_maps, list(range(N_CORES)))
    out = np.zeros((B, T, D), dtype=np.float32)
    for c in range(N_CORES):
        out[c // 4] += res.results[c]["out"].astype(np.float32)
    return out
